# revision 6
# baseline (speedup 1.0000x reference)
"""Trainium2 Bass kernel for 16-head dense multi-head attention.

Problem: B=2, S=2048, d_model=1024, 16 heads (head dim 64), fp32.
Sharding over 8 NeuronCores: core c -> batch b = c//4, head-group g = c%4
(4 heads, i.e. columns 256g:256(g+1) of Wq/Wk/Wv and rows 256g:256(g+1)
of W0). Each core computes a partial output projection for its batch;
the host sums the 4 partials per batch.

Per-core design (everything stays transposed so softmax denominators come
free):
  - X^T materialized in SBUF via PE transposes (fp32 exact), 4 blocks per
    PSUM bank so the PSUM->SBUF copy runs at [128,512] granularity.
  - Q^T, K^T projections with W tiles stationary, X^T moving.
  - scores^T for a head PAIR computed with row-group-disjoint matmuls
    (heads live in partitions 0:64 / 64:128 of the Q^T/K^T tiles), which
    the PE runs concurrently via its LDWEIGHTS reorder window.
  - exp via ACT directly out of PSUM over [128,1024] supertiles
    (scale=1/8 folded into the activation's free affine).
  - V augmented with a ones column -> P @ [V|1] gives both the head output
    and the softmax denominator in one PSUM accumulation (M=65).
  - reciprocal of the denominator broadcast across partitions with a K=1
    PE matmul; normalization fused into the PSUM->SBUF copy (DVE multiply)
    and emission-deferred so the PE never stalls on the DVE chain.
  - output projection accumulates into a [128,1024] PSUM supertile.
All big matmuls run in float32r (single-pass PE mode, ~1.5e-4 rel error).
"""

import os
import sys

import numpy as np

if "/opt/trn_rl_repo" not in sys.path:
    sys.path.insert(0, "/opt/trn_rl_repo")

S = 2048
DM = 1024
DKL = 256  # local q/k/v width (4 heads x 64)
HD = 64
NHL = 4  # heads per core
NST = S // 128  # 16 s-tiles
NC_ = DM // 128  # 8 contraction tiles
NQC = 4  # sq chunks of 512
GROUPS2 = [(g, 2) for g in range(0, 16, 2)]  # 8 groups of 2 sk-tiles

_CACHE = {}
LAST_EXEC_NS = None


def _maybe_enable_trace():
    try:
        import antenv

        p = "/opt/trn_rl_repo/antenv"
        if p not in antenv.__path__:
            antenv.__path__.append(p)
        from antenv.axon_hooks import (
            get_axon_ntff_profile_hook,
            set_axon_ntff_profile_hook,
        )

        if get_axon_ntff_profile_hook() is None:
            from trn_agent_boot.trn_boot import _ntff_profile_via_ctypes

            set_axon_ntff_profile_hook(
                _ntff_profile_via_ctypes("/opt/axon/libaxon_pjrt.so")
            )
        return True
    except Exception:
        return False


def _build_nc():
    import concourse.tile as tile
    from concourse import bacc, mybir
    from concourse.masks import make_identity

    f32 = mybir.dt.float32
    f32r = mybir.dt.float32r
    EXP = mybir.ActivationFunctionType.Exp

    nc = bacc.Bacc("TRN2", target_bir_lowering=False, debug=False, num_devices=8)
    x = nc.dram_tensor("x", [S, DM], f32, kind="ExternalInput").ap()
    wq_d = nc.dram_tensor("wq", [DM, DKL], f32, kind="ExternalInput").ap()
    wk_d = nc.dram_tensor("wk", [DM, DKL], f32, kind="ExternalInput").ap()
    wv_d = nc.dram_tensor("wv", [DM, DKL], f32, kind="ExternalInput").ap()
    w0_d = nc.dram_tensor("w0", [DKL, DM], f32, kind="ExternalInput").ap()
    y = nc.dram_tensor("y", [S, DM], f32, kind="ExternalOutput").ap()

    with tile.TileContext(nc) as tc:
        with (
            tc.tile_pool(name="const", bufs=1) as const_pool,
            tc.tile_pool(name="wpool", bufs=1) as w_pool,
            tc.tile_pool(name="qkv", bufs=1) as qkv_pool,
        ):
            identity = const_pool.tile([128, 128], f32)
            make_identity(nc, identity)
            ones_f = const_pool.tile([1, HD], f32)
            nc.vector.memset(ones_f, 1.0)
            ones_t = const_pool.tile([1, HD], f32r)
            nc.vector.tensor_copy(ones_t[:], ones_f[:])
            ones_col = const_pool.tile([128, NST, NHL, 1], f32)
            nc.vector.memset(ones_col, 1.0)

            wq_sb = w_pool.tile([128, NC_, DKL], f32r)
            wk_sb = w_pool.tile([128, NC_, DKL], f32r)
            wv_sb = w_pool.tile([128, NC_, DKL], f32r)
            w0_sb = w_pool.tile([128, DKL // 128, DM], f32r)

            # Persistent transposed activations / projections.
            qt = [qkv_pool.tile([128, S], f32r, name=f"qt{i}") for i in range(2)]
            kt = [qkv_pool.tile([128, S], f32r, name=f"kt{i}") for i in range(2)]
            vaug = qkv_pool.tile([128, NST, NHL, HD + 1], f32r)
            nc.vector.tensor_copy(vaug[:, :, :, HD : HD + 1], ones_col[:])

            # ---------------- Phase B: X^T + Q/K/V projections -------------
            with (
                tc.tile_pool(name="xs", bufs=10) as xs_pool,
                tc.tile_pool(name="xt", bufs=1) as xt_pool,
                tc.tile_pool(name="tp_ps", bufs=4, space="PSUM") as tp_pool,
                tc.tile_pool(name="proj_ps", bufs=3, space="PSUM") as proj_pool,
            ):
                xt = xt_pool.tile([128, NC_, S], f32r)
                xs_tiles = {}
                for st in range(2):
                    xs_tiles[st] = xs_pool.tile([128, DM], f32, tag="xs", name=f"xs{st}")
                    nc.sync.dma_start(
                        out=xs_tiles[st], in_=x[st * 128 : (st + 1) * 128, :]
                    )
                for dst2, src2 in (
                    (wq_sb, wq_d),
                    (wk_sb, wk_d),
                    (wv_sb, wv_d),
                    (w0_sb, w0_d),
                ):
                    nc.sync.dma_start(
                        out=dst2,
                        in_=src2.bitcast(f32r).rearrange("(t p) c -> p t c", p=128),
                    )
                for st in range(NST):
                    if st in xs_tiles:
                        xs = xs_tiles[st]
                    else:
                        xs = xs_pool.tile([128, DM], f32, tag="xs", name=f"xs{st}")
                        nc.sync.dma_start(
                            out=xs, in_=x[st * 128 : (st + 1) * 128, :]
                        )
                    ssl = slice(st * 128, (st + 1) * 128)
                    for cg in range(2):  # groups of 4 contraction tiles
                        tp = tp_pool.tile([128, 512], f32, tag="tp")
                        for c4 in range(4):
                            c = cg * 4 + c4
                            nc.tensor.transpose(
                                tp[:, c4 * 128 : (c4 + 1) * 128],
                                xs[:, c * 128 : (c + 1) * 128],
                                identity,
                            )
                        nc.vector.tensor_copy(
                            xt[:, cg * 4 : (cg + 1) * 4, ssl],
                            tp[:].rearrange("p (c s) -> p c s", c=4),
                        )
                    # V projection for this s-tile (natural layout).
                    vps = proj_pool.tile([128, DKL], f32, tag="proj")
                    for c in range(NC_):
                        nc.tensor.matmul(
                            vps[:],
                            lhsT=xt[:, c, ssl],
                            rhs=wv_sb[:, c, :],
                            start=(c == 0),
                            stop=(c == NC_ - 1),
                            skip_group_check=True,
                        )
                    nc.vector.tensor_copy(
                        vaug[:, st, :, 0:HD],
                        vps[:].rearrange("p (h d) -> p h d", h=NHL),
                    )
                    # After each 512-row span: Q^T / K^T blocks for that span.
                    if st % 4 == 3:
                        q2 = st // 4
                        sl = slice(q2 * 512, (q2 + 1) * 512)
                        for wsb, dst in ((wq_sb, qt), (wk_sb, kt)):
                            for j in range(2):
                                ps = proj_pool.tile([128, 512], f32, tag="proj")
                                for c in range(NC_):
                                    nc.tensor.matmul(
                                        ps[:],
                                        lhsT=wsb[:, c, j * 128 : (j + 1) * 128],
                                        rhs=xt[:, c, sl],
                                        start=(c == 0),
                                        stop=(c == NC_ - 1),
                                        skip_group_check=True,
                                    )
                                nc.vector.tensor_copy(dst[j][:, sl], ps[:])

            # ---------------- Phase C: attention + output ------------------
            with (
                tc.tile_pool(name="sc_ps", bufs=2, space="PSUM") as sc_pool,
                tc.tile_pool(name="mm4_ps", bufs=2, space="PSUM") as mm4_pool,
                tc.tile_pool(name="mm5_ps", bufs=1, space="PSUM") as mm5_pool,
                tc.tile_pool(name="pt", bufs=6) as pt_pool,
                tc.tile_pool(name="cc", bufs=4) as cc_pool,
                tc.tile_pool(name="rb", bufs=3) as rb_pool,
                tc.tile_pool(name="yo", bufs=3) as yo_pool,
            ):

                def emit_norm(h, mm4ps, c_tiles):
                    """concat^T[h rows] = heads^T / rowsum (deferred)."""
                    pt_i = h // 2
                    hp = (h % 2) * HD
                    recip_f = rb_pool.tile([1, 512], f32, tag="recipf", name=f"rcf{h}")
                    nc.vector.reciprocal(recip_f[:], mm4ps[HD : HD + 1, :])
                    recip = rb_pool.tile([1, 512], f32r, tag="recip", name=f"rc{h}")
                    nc.vector.tensor_copy(recip[:], recip_f[:])
                    bc = mm5_pool.tile([HD, 512], f32, tag="mm5", name=f"bc{h}")
                    nc.tensor.matmul(
                        bc[:],
                        lhsT=ones_t[:],
                        rhs=recip[:],
                        start=True,
                        stop=True,
                        skip_group_check=True,
                    )
                    rbt = rb_pool.tile([HD, 512], f32, tag="rb", name=f"rb{h}")
                    nc.vector.tensor_copy(rbt[:], bc[:])
                    nc.vector.tensor_mul(
                        c_tiles[pt_i][hp : hp + HD, :], mm4ps[0:HD, :], rbt[:]
                    )

                for q in range(NQC):
                    qsl = slice(q * 512, (q + 1) * 512)
                    c_tiles = [
                        cc_pool.tile([128, 512], f32r, tag="cc", name=f"cc{q}_{i}")
                        for i in range(2)
                    ]
                    pending = []
                    for pi in range(2):  # head pairs (0,1) and (2,3)
                        h0, h1 = 2 * pi, 2 * pi + 1
                        mm4 = [
                            mm4_pool.tile(
                                [HD + 1, 512], f32, tag="mm4", name=f"mm4_{q}_{h}"
                            )
                            for h in (h0, h1)
                        ]

                        def emit_mm4(hi, pt_g, g0, glen, mm4=mm4, pi=pi):
                            for j in range(glen):
                                t = g0 + j
                                nc.tensor.matmul(
                                    mm4[hi][:],
                                    lhsT=vaug[:, t, 2 * pi + hi, :],
                                    rhs=pt_g[hi][:, j * 512 : (j + 1) * 512],
                                    start=(t == 0),
                                    stop=(t == NST - 1),
                                    skip_group_check=True,
                                )

                        prev = None
                        for gi, (g0, glen) in enumerate(GROUPS2):
                            sc = [
                                sc_pool.tile(
                                    [128, glen * 512],
                                    f32,
                                    tag="sc",
                                    name=f"sc{q}_{pi}_{gi}_{i}",
                                )
                                for i in range(2)
                            ]
                            for j in range(glen):
                                t = g0 + j
                                tsl = slice(t * 128, (t + 1) * 128)
                                osl = slice(j * 512, (j + 1) * 512)
                                for hi, hpp in ((0, 0), (1, HD)):
                                    nc.tensor.matmul(
                                        sc[hi][:, osl],
                                        lhsT=kt[pi][hpp : hpp + HD, tsl],
                                        rhs=qt[pi][hpp : hpp + HD, qsl],
                                        start=True,
                                        stop=True,
                                        skip_group_check=True,
                                    )
                            pt_g = [
                                pt_pool.tile(
                                    [128, glen * 512],
                                    f32r,
                                    tag="pt",
                                    name=f"pt{q}_{pi}_{gi}_{i}",
                                )
                                for i in range(2)
                            ]
                            for hi in range(2):
                                nc.scalar.activation(
                                    out=pt_g[hi][:],
                                    in_=sc[hi][:],
                                    func=EXP,
                                    scale=0.125,
                                )
                            if prev is not None:
                                for hi in range(2):
                                    emit_mm4(hi, *prev)
                            if gi == 1 and pending:
                                for h, m in pending:
                                    emit_norm(h, m, c_tiles)
                                pending = []
                            prev = (pt_g, g0, glen)
                        for hi in range(2):
                            emit_mm4(hi, *prev)
                        pending = [(h0, mm4[0]), (h1, mm4[1])]
                    for h, m in pending:
                        emit_norm(h, m, c_tiles)

                    # Output projection for this 512-row span.
                    for sub in range(4):
                        ps5 = mm5_pool.tile(
                            [128, DM], f32, tag="mm5", name=f"ps5_{q}_{sub}"
                        )
                        for nh2 in range(2):
                            for c2 in range(2):
                                nc.tensor.matmul(
                                    ps5[:, nh2 * 512 : (nh2 + 1) * 512],
                                    lhsT=c_tiles[c2][:, sub * 128 : (sub + 1) * 128],
                                    rhs=w0_sb[:, c2, nh2 * 512 : (nh2 + 1) * 512],
                                    start=(c2 == 0),
                                    stop=(c2 == 1),
                                    skip_group_check=True,
                                )
                        yo = yo_pool.tile([128, DM], f32, tag="yo")
                        nc.vector.tensor_copy(yo[:], ps5[:])
                        r0 = q * 512 + sub * 128
                        nc.sync.dma_start(out=y[r0 : r0 + 128, :], in_=yo[:])

    nc.compile()
    return nc


def _get_nc():
    if "nc" not in _CACHE:
        _CACHE["nc"] = _build_nc()
    return _CACHE["nc"]


def kernel(inputs, Wq, Wk, Wv, W0):
    global LAST_EXEC_NS
    from concourse.bass_utils import run_bass_kernel_spmd

    inputs = np.asarray(inputs, dtype=np.float32)
    Wq = np.asarray(Wq, dtype=np.float32)
    Wk = np.asarray(Wk, dtype=np.float32)
    Wv = np.asarray(Wv, dtype=np.float32)
    W0 = np.asarray(W0, dtype=np.float32)

    trace = bool(os.environ.get("BASS_KERNEL_TRACE"))
    if trace:
        trace = _maybe_enable_trace()

    nc = _get_nc()
    in_maps = []
    for c in range(8):
        b, g = divmod(c, 4)
        sl = slice(DKL * g, DKL * (g + 1))
        in_maps.append(
            {
                "x": np.ascontiguousarray(inputs[b]),
                "wq": np.ascontiguousarray(Wq[:, sl]),
                "wk": np.ascontiguousarray(Wk[:, sl]),
                "wv": np.ascontiguousarray(Wv[:, sl]),
                "w0": np.ascontiguousarray(W0[sl, :]),
            }
        )
    res = run_bass_kernel_spmd(nc, in_maps, list(range(8)), trace=trace)
    LAST_EXEC_NS = res.exec_time_ns
    outs = [res.results[i]["y"] for i in range(8)]
    out = np.stack(
        [
            outs[0] + outs[1] + outs[2] + outs[3],
            outs[4] + outs[5] + outs[6] + outs[7],
        ]
    )
    return out.astype(np.float32)


# revision 8
# speedup vs baseline: 1.0306x; 1.0306x over previous
"""Trainium2 Bass kernel for 16-head dense multi-head attention.

Problem: B=2, S=2048, d_model=1024, 16 heads (head dim 64), fp32.
Sharding over 8 NeuronCores: core c -> batch b = c//4, head-group g = c%4
(4 heads, i.e. columns 256g:256(g+1) of Wq/Wk/Wv and rows 256g:256(g+1)
of W0). Each core computes a partial output projection for its batch;
the host sums the 4 partials per batch.

Per-core design (everything stays transposed so softmax denominators come
free):
  - X^T materialized in SBUF via PE transposes (fp32 exact), 4 blocks per
    PSUM bank so the PSUM->SBUF copy runs at [128,512] granularity.
  - Q^T, K^T projections with W tiles stationary, X^T moving.
  - scores^T for a head PAIR computed with row-group-disjoint matmuls
    (heads live in partitions 0:64 / 64:128 of the Q^T/K^T tiles), which
    the PE runs concurrently via its LDWEIGHTS reorder window.
  - exp via ACT directly out of PSUM over [128,1024] supertiles
    (scale=1/8 folded into the activation's free affine).
  - V augmented with a ones column -> P @ [V|1] gives both the head output
    and the softmax denominator in one PSUM accumulation (M=65).
  - reciprocal of the denominator broadcast across partitions with a K=1
    PE matmul; normalization fused into the PSUM->SBUF copy (DVE multiply)
    and emission-deferred so the PE never stalls on the DVE chain.
  - output projection accumulates into a [128,1024] PSUM supertile.
All big matmuls run in float32r (single-pass PE mode, ~1.5e-4 rel error).
"""

import os
import sys

import numpy as np

if "/opt/trn_rl_repo" not in sys.path:
    sys.path.insert(0, "/opt/trn_rl_repo")

S = 2048
DM = 1024
DKL = 256  # local q/k/v width (4 heads x 64)
HD = 64
NHL = 4  # heads per core
NST = S // 128  # 16 s-tiles
NC_ = DM // 128  # 8 contraction tiles
NQC = 4  # sq chunks of 512
GROUPS2 = [(g, 2) for g in range(0, 16, 2)]  # 8 groups of 2 sk-tiles

_CACHE = {}
LAST_EXEC_NS = None


def _maybe_enable_trace():
    try:
        import antenv

        p = "/opt/trn_rl_repo/antenv"
        if p not in antenv.__path__:
            antenv.__path__.append(p)
        from antenv.axon_hooks import (
            get_axon_ntff_profile_hook,
            set_axon_ntff_profile_hook,
        )

        if get_axon_ntff_profile_hook() is None:
            from trn_agent_boot.trn_boot import _ntff_profile_via_ctypes

            set_axon_ntff_profile_hook(
                _ntff_profile_via_ctypes("/opt/axon/libaxon_pjrt.so")
            )
        return True
    except Exception:
        return False


def _build_nc():
    import concourse.tile as tile
    from concourse import bacc, mybir
    from concourse.masks import make_identity

    f32 = mybir.dt.float32
    f32r = getattr(mybir.dt, os.environ.get("MM_DTYPE", "float32r"))
    EXP = mybir.ActivationFunctionType.Exp

    nc = bacc.Bacc("TRN2", target_bir_lowering=False, debug=False, num_devices=8)
    x = nc.dram_tensor("x", [S, DM], f32, kind="ExternalInput").ap()
    wq_d = nc.dram_tensor("wq", [DM, DKL], f32, kind="ExternalInput").ap()
    wk_d = nc.dram_tensor("wk", [DM, DKL], f32, kind="ExternalInput").ap()
    wv_d = nc.dram_tensor("wv", [DM, DKL], f32, kind="ExternalInput").ap()
    w0_d = nc.dram_tensor("w0", [DKL, DM], f32, kind="ExternalInput").ap()
    y = nc.dram_tensor("y", [S, DM], f32, kind="ExternalOutput").ap()

    with tile.TileContext(nc) as tc:
        with (
            tc.tile_pool(name="const", bufs=1) as const_pool,
            tc.tile_pool(name="wpool", bufs=1) as w_pool,
            tc.tile_pool(name="qkv", bufs=1) as qkv_pool,
        ):
            identity = const_pool.tile([128, 128], f32)
            make_identity(nc, identity)
            ones_f = const_pool.tile([1, HD], f32)
            nc.vector.memset(ones_f, 1.0)
            ones_t = const_pool.tile([1, HD], f32r)
            nc.vector.tensor_copy(ones_t[:], ones_f[:])
            ones_col = const_pool.tile([128, NST, NHL, 1], f32)
            nc.vector.memset(ones_col, 1.0)

            wq_sb = w_pool.tile([128, NC_, DKL], f32r)
            wk_sb = w_pool.tile([128, NC_, DKL], f32r)
            wv_sb = w_pool.tile([128, NC_, DKL], f32r)
            w0_sb = w_pool.tile([128, DKL // 128, DM], f32r)

            # Persistent transposed activations / projections.
            qt = [qkv_pool.tile([128, S], f32r, name=f"qt{i}") for i in range(2)]
            kt = [qkv_pool.tile([128, S], f32r, name=f"kt{i}") for i in range(2)]
            vaug = qkv_pool.tile([128, NST, NHL, HD + 1], f32r)
            nc.vector.tensor_copy(vaug[:, :, :, HD : HD + 1], ones_col[:])

            # ---------------- Phase B: X^T + Q/K/V projections -------------
            with (
                tc.tile_pool(name="xs", bufs=10) as xs_pool,
                tc.tile_pool(name="xt", bufs=1) as xt_pool,
                tc.tile_pool(name="tp_ps", bufs=4, space="PSUM") as tp_pool,
                tc.tile_pool(name="proj_ps", bufs=3, space="PSUM") as proj_pool,
            ):
                xt = xt_pool.tile([128, NC_, S], f32r)
                xs_tiles = {}
                for st in range(2):
                    xs_tiles[st] = xs_pool.tile([128, DM], f32, tag="xs", name=f"xs{st}")
                    nc.sync.dma_start(
                        out=xs_tiles[st], in_=x[st * 128 : (st + 1) * 128, :]
                    )
                for wi, (dst2, src2) in enumerate(
                    (
                        (wq_sb, wq_d),
                        (wk_sb, wk_d),
                        (wv_sb, wv_d),
                        (w0_sb, w0_d),
                    )
                ):
                    if f32r == mybir.dt.float32r:
                        nc.sync.dma_start(
                            out=dst2,
                            in_=src2.bitcast(f32r).rearrange(
                                "(t p) c -> p t c", p=128
                            ),
                        )
                    else:
                        wst = xs_pool.tile(
                            [128, dst2.shape[1], dst2.shape[2]],
                            f32,
                            tag="xs",
                            name=f"wst{wi}",
                        )
                        nc.sync.dma_start(
                            out=wst,
                            in_=src2.rearrange("(t p) c -> p t c", p=128),
                        )
                        nc.vector.tensor_copy(dst2[:], wst[:])
                for st in range(NST):
                    if st in xs_tiles:
                        xs = xs_tiles[st]
                    else:
                        xs = xs_pool.tile([128, DM], f32, tag="xs", name=f"xs{st}")
                        nc.sync.dma_start(
                            out=xs, in_=x[st * 128 : (st + 1) * 128, :]
                        )
                    ssl = slice(st * 128, (st + 1) * 128)
                    for cg in range(2):  # groups of 4 contraction tiles
                        tp = tp_pool.tile([128, 512], f32, tag="tp")
                        for c4 in range(4):
                            c = cg * 4 + c4
                            nc.tensor.transpose(
                                tp[:, c4 * 128 : (c4 + 1) * 128],
                                xs[:, c * 128 : (c + 1) * 128],
                                identity,
                            )
                        nc.vector.tensor_copy(
                            xt[:, cg * 4 : (cg + 1) * 4, ssl],
                            tp[:].rearrange("p (c s) -> p c s", c=4),
                        )
                    # V projection for this s-tile (natural layout).
                    vps = proj_pool.tile([128, DKL], f32, tag="proj")
                    for c in range(NC_):
                        nc.tensor.matmul(
                            vps[:],
                            lhsT=xt[:, c, ssl],
                            rhs=wv_sb[:, c, :],
                            start=(c == 0),
                            stop=(c == NC_ - 1),
                            skip_group_check=True,
                        )
                    nc.vector.tensor_copy(
                        vaug[:, st, :, 0:HD],
                        vps[:].rearrange("p (h d) -> p h d", h=NHL),
                    )
                    # After each 512-row span: Q^T / K^T blocks for that span.
                    if st % 4 == 3:
                        q2 = st // 4
                        sl = slice(q2 * 512, (q2 + 1) * 512)
                        for wsb, dst in ((wq_sb, qt), (wk_sb, kt)):
                            for j in range(2):
                                ps = proj_pool.tile([128, 512], f32, tag="proj")
                                for c in range(NC_):
                                    nc.tensor.matmul(
                                        ps[:],
                                        lhsT=wsb[:, c, j * 128 : (j + 1) * 128],
                                        rhs=xt[:, c, sl],
                                        start=(c == 0),
                                        stop=(c == NC_ - 1),
                                        skip_group_check=True,
                                    )
                                nc.vector.tensor_copy(dst[j][:, sl], ps[:])

            # ---------------- Phase C: attention + output ------------------
            with (
                tc.tile_pool(name="sc_ps", bufs=2, space="PSUM") as sc_pool,
                tc.tile_pool(name="mm4_ps", bufs=2, space="PSUM") as mm4_pool,
                tc.tile_pool(name="mm5_ps", bufs=1, space="PSUM") as mm5_pool,
                tc.tile_pool(name="pt", bufs=6) as pt_pool,
                tc.tile_pool(name="cc", bufs=4) as cc_pool,
                tc.tile_pool(name="rb", bufs=3) as rb_pool,
                tc.tile_pool(name="yo", bufs=3) as yo_pool,
            ):

                def emit_norm(h, mm4ps, c_tiles):
                    """concat^T[h rows] = heads^T / rowsum (deferred)."""
                    pt_i = h // 2
                    hp = (h % 2) * HD
                    recip_f = rb_pool.tile([1, 512], f32, tag="recipf", name=f"rcf{h}")
                    nc.vector.reciprocal(recip_f[:], mm4ps[HD : HD + 1, :])
                    recip = rb_pool.tile([1, 512], f32r, tag="recip", name=f"rc{h}")
                    nc.vector.tensor_copy(recip[:], recip_f[:])
                    bc = mm5_pool.tile([HD, 512], f32, tag="mm5", name=f"bc{h}")
                    nc.tensor.matmul(
                        bc[:],
                        lhsT=ones_t[:],
                        rhs=recip[:],
                        start=True,
                        stop=True,
                        skip_group_check=True,
                    )
                    rbt = rb_pool.tile([HD, 512], f32, tag="rb", name=f"rb{h}")
                    nc.vector.tensor_copy(rbt[:], bc[:])
                    nc.vector.tensor_mul(
                        c_tiles[pt_i][hp : hp + HD, :], mm4ps[0:HD, :], rbt[:]
                    )

                for q in range(NQC):
                    qsl = slice(q * 512, (q + 1) * 512)
                    c_tiles = [
                        cc_pool.tile([128, 512], f32r, tag="cc", name=f"cc{q}_{i}")
                        for i in range(2)
                    ]
                    pending = []
                    for pi in range(2):  # head pairs (0,1) and (2,3)
                        h0, h1 = 2 * pi, 2 * pi + 1
                        mm4 = [
                            mm4_pool.tile(
                                [HD + 1, 512], f32, tag="mm4", name=f"mm4_{q}_{h}"
                            )
                            for h in (h0, h1)
                        ]

                        def emit_mm4(hi, pt_g, g0, glen, mm4=mm4, pi=pi):
                            for j in range(glen):
                                t = g0 + j
                                nc.tensor.matmul(
                                    mm4[hi][:],
                                    lhsT=vaug[:, t, 2 * pi + hi, :],
                                    rhs=pt_g[hi][:, j * 512 : (j + 1) * 512],
                                    start=(t == 0),
                                    stop=(t == NST - 1),
                                    skip_group_check=True,
                                )

                        prev = None
                        for gi, (g0, glen) in enumerate(GROUPS2):
                            sc = [
                                sc_pool.tile(
                                    [128, glen * 512],
                                    f32,
                                    tag="sc",
                                    name=f"sc{q}_{pi}_{gi}_{i}",
                                )
                                for i in range(2)
                            ]
                            for j in range(glen):
                                t = g0 + j
                                tsl = slice(t * 128, (t + 1) * 128)
                                osl = slice(j * 512, (j + 1) * 512)
                                for hi, hpp in ((0, 0), (1, HD)):
                                    nc.tensor.matmul(
                                        sc[hi][:, osl],
                                        lhsT=kt[pi][hpp : hpp + HD, tsl],
                                        rhs=qt[pi][hpp : hpp + HD, qsl],
                                        start=True,
                                        stop=True,
                                        skip_group_check=True,
                                    )
                            pt_g = [
                                pt_pool.tile(
                                    [128, glen * 512],
                                    f32r,
                                    tag="pt",
                                    name=f"pt{q}_{pi}_{gi}_{i}",
                                )
                                for i in range(2)
                            ]
                            for hi in range(2):
                                nc.scalar.activation(
                                    out=pt_g[hi][:],
                                    in_=sc[hi][:],
                                    func=EXP,
                                    scale=0.125,
                                )
                            if prev is not None:
                                for hi in range(2):
                                    emit_mm4(hi, *prev)
                            if gi == 1 and pending:
                                for h, m in pending:
                                    emit_norm(h, m, c_tiles)
                                pending = []
                            prev = (pt_g, g0, glen)
                        for hi in range(2):
                            emit_mm4(hi, *prev)
                        pending = [(h0, mm4[0]), (h1, mm4[1])]
                    for h, m in pending:
                        emit_norm(h, m, c_tiles)

                    # Output projection for this 512-row span.
                    for sub in range(4):
                        ps5 = mm5_pool.tile(
                            [128, DM], f32, tag="mm5", name=f"ps5_{q}_{sub}"
                        )
                        for nh2 in range(2):
                            for c2 in range(2):
                                nc.tensor.matmul(
                                    ps5[:, nh2 * 512 : (nh2 + 1) * 512],
                                    lhsT=c_tiles[c2][:, sub * 128 : (sub + 1) * 128],
                                    rhs=w0_sb[:, c2, nh2 * 512 : (nh2 + 1) * 512],
                                    start=(c2 == 0),
                                    stop=(c2 == 1),
                                    skip_group_check=True,
                                )
                        yo = yo_pool.tile([128, DM], f32, tag="yo")
                        nc.vector.tensor_copy(yo[:], ps5[:])
                        r0 = q * 512 + sub * 128
                        nc.sync.dma_start(out=y[r0 : r0 + 128, :], in_=yo[:])

    nc.compile()
    return nc


def _get_nc():
    if "nc" not in _CACHE:
        _CACHE["nc"] = _build_nc()
    return _CACHE["nc"]


def kernel(inputs, Wq, Wk, Wv, W0):
    global LAST_EXEC_NS
    from concourse.bass_utils import run_bass_kernel_spmd

    inputs = np.asarray(inputs, dtype=np.float32)
    Wq = np.asarray(Wq, dtype=np.float32)
    Wk = np.asarray(Wk, dtype=np.float32)
    Wv = np.asarray(Wv, dtype=np.float32)
    W0 = np.asarray(W0, dtype=np.float32)

    trace = bool(os.environ.get("BASS_KERNEL_TRACE"))
    if trace:
        trace = _maybe_enable_trace()

    nc = _get_nc()
    in_maps = []
    for c in range(8):
        b, g = divmod(c, 4)
        sl = slice(DKL * g, DKL * (g + 1))
        in_maps.append(
            {
                "x": np.ascontiguousarray(inputs[b]),
                "wq": np.ascontiguousarray(Wq[:, sl]),
                "wk": np.ascontiguousarray(Wk[:, sl]),
                "wv": np.ascontiguousarray(Wv[:, sl]),
                "w0": np.ascontiguousarray(W0[sl, :]),
            }
        )
    res = run_bass_kernel_spmd(nc, in_maps, list(range(8)), trace=trace)
    LAST_EXEC_NS = res.exec_time_ns
    outs = [res.results[i]["y"] for i in range(8)]
    out = np.stack(
        [
            outs[0] + outs[1] + outs[2] + outs[3],
            outs[4] + outs[5] + outs[6] + outs[7],
        ]
    )
    return out.astype(np.float32)


# revision 10
# speedup vs baseline: 1.1193x; 1.0861x over previous
"""Trainium2 Bass kernel for 16-head dense multi-head attention.

Problem: B=2, S=2048, d_model=1024, 16 heads (head dim 64), fp32.
Sharding over 8 NeuronCores: core c -> batch b = c//4, head-group g = c%4
(4 heads, i.e. columns 256g:256(g+1) of Wq/Wk/Wv and rows 256g:256(g+1)
of W0). Each core computes a partial output projection for its batch;
the host sums the 4 partials per batch.

Per-core design (everything stays transposed so softmax denominators come
free):
  - X^T materialized in SBUF via PE transposes (fp32 exact), 4 blocks per
    PSUM bank so the PSUM->SBUF copy runs at [128,512] granularity.
  - Q^T, K^T projections with W tiles stationary, X^T moving.
  - scores^T for a head PAIR computed with row-group-disjoint matmuls
    (heads live in partitions 0:64 / 64:128 of the Q^T/K^T tiles), which
    the PE runs concurrently via its LDWEIGHTS reorder window.
  - exp via ACT directly out of PSUM over [128,1024] supertiles
    (scale=1/8 folded into the activation's free affine).
  - V augmented with a ones column -> P @ [V|1] gives both the head output
    and the softmax denominator in one PSUM accumulation (M=65).
  - reciprocal of the denominator broadcast across partitions with a K=1
    PE matmul; normalization fused into the PSUM->SBUF copy (DVE multiply)
    and emission-deferred so the PE never stalls on the DVE chain.
  - output projection accumulates into a [128,1024] PSUM supertile.
All big matmuls run in float32r (single-pass PE mode, ~1.5e-4 rel error).
"""

import os
import sys

import numpy as np

if "/opt/trn_rl_repo" not in sys.path:
    sys.path.insert(0, "/opt/trn_rl_repo")

S = 2048
DM = 1024
DKL = 256  # local q/k/v width (4 heads x 64)
HD = 64
NHL = 4  # heads per core
NST = S // 128  # 16 s-tiles
NC_ = DM // 128  # 8 contraction tiles
NQC = 4  # sq chunks of 512
GROUPS2 = [(g, 2) for g in range(0, 16, 2)]  # 8 groups of 2 sk-tiles

_CACHE = {}
LAST_EXEC_NS = None


def _maybe_enable_trace():
    try:
        import antenv

        p = "/opt/trn_rl_repo/antenv"
        if p not in antenv.__path__:
            antenv.__path__.append(p)
        from antenv.axon_hooks import (
            get_axon_ntff_profile_hook,
            set_axon_ntff_profile_hook,
        )

        if get_axon_ntff_profile_hook() is None:
            from trn_agent_boot.trn_boot import _ntff_profile_via_ctypes

            set_axon_ntff_profile_hook(
                _ntff_profile_via_ctypes("/opt/axon/libaxon_pjrt.so")
            )
        return True
    except Exception:
        return False


def _build_nc():
    import concourse.tile as tile
    from concourse import bacc, mybir
    from concourse.masks import make_identity

    f32 = mybir.dt.float32
    f32r = getattr(mybir.dt, os.environ.get("MM_DTYPE", "float32r"))
    EXP = mybir.ActivationFunctionType.Exp

    nc = bacc.Bacc("TRN2", target_bir_lowering=False, debug=False, num_devices=8)
    x = nc.dram_tensor("x", [S, DM], f32, kind="ExternalInput").ap()
    wq_d = nc.dram_tensor("wq", [DM, DKL], f32, kind="ExternalInput").ap()
    wk_d = nc.dram_tensor("wk", [DM, DKL], f32, kind="ExternalInput").ap()
    wv_d = nc.dram_tensor("wv", [DM, DKL], f32, kind="ExternalInput").ap()
    w0_d = nc.dram_tensor("w0", [DKL, DM], f32, kind="ExternalInput").ap()
    y = nc.dram_tensor("y", [S, DM], f32, kind="ExternalOutput").ap()

    with tile.TileContext(nc) as tc:
        with (
            tc.tile_pool(name="const", bufs=1) as const_pool,
            tc.tile_pool(name="wpool", bufs=1) as w_pool,
            tc.tile_pool(name="qkv", bufs=1) as qkv_pool,
        ):
            identity = const_pool.tile([128, 128], f32)
            make_identity(nc, identity)
            ones_f = const_pool.tile([1, HD], f32)
            nc.vector.memset(ones_f, 1.0)
            ones_t = const_pool.tile([1, HD], f32r)
            nc.vector.tensor_copy(ones_t[:], ones_f[:])
            ones_col = const_pool.tile([128, NST, NHL, 1], f32)
            nc.vector.memset(ones_col, 1.0)

            wq_sb = w_pool.tile([128, NC_, DKL], f32r)
            wk_sb = w_pool.tile([128, NC_, DKL], f32r)
            wv_sb = w_pool.tile([128, NC_, DKL], f32r)
            w0_sb = w_pool.tile([128, DKL // 128, DM], f32r)

            # Persistent transposed activations / projections.
            qt = [qkv_pool.tile([128, S], f32r, name=f"qt{i}") for i in range(2)]
            kt = [qkv_pool.tile([128, S], f32r, name=f"kt{i}") for i in range(2)]
            vaug = qkv_pool.tile([128, NST, NHL, HD + 1], f32r)
            nc.vector.tensor_copy(vaug[:, :, :, HD : HD + 1], ones_col[:])

            # ---------------- Phase B: X^T + Q/K/V projections -------------
            with (
                tc.tile_pool(name="xs", bufs=10) as xs_pool,
                tc.tile_pool(name="xt", bufs=1) as xt_pool,
                tc.tile_pool(name="tp_ps", bufs=4, space="PSUM") as tp_pool,
                tc.tile_pool(name="proj_ps", bufs=3, space="PSUM") as proj_pool,
            ):
                xt = xt_pool.tile([128, NC_, S], f32r)
                xs_tiles = {}
                for st in range(2):
                    xs_tiles[st] = xs_pool.tile([128, DM], f32, tag="xs", name=f"xs{st}")
                    nc.sync.dma_start(
                        out=xs_tiles[st], in_=x[st * 128 : (st + 1) * 128, :]
                    )
                for wi, (dst2, src2) in enumerate(
                    (
                        (wq_sb, wq_d),
                        (wk_sb, wk_d),
                        (wv_sb, wv_d),
                        (w0_sb, w0_d),
                    )
                ):
                    if f32r == mybir.dt.float32r:
                        nc.sync.dma_start(
                            out=dst2,
                            in_=src2.bitcast(f32r).rearrange(
                                "(t p) c -> p t c", p=128
                            ),
                        )
                    else:
                        wst = xs_pool.tile(
                            [128, dst2.shape[1], dst2.shape[2]],
                            f32,
                            tag="xs",
                            name=f"wst{wi}",
                        )
                        nc.sync.dma_start(
                            out=wst,
                            in_=src2.rearrange("(t p) c -> p t c", p=128),
                        )
                        nc.vector.tensor_copy(dst2[:], wst[:])
                for st in range(NST):
                    if st in xs_tiles:
                        xs = xs_tiles[st]
                    else:
                        xs = xs_pool.tile([128, DM], f32, tag="xs", name=f"xs{st}")
                        nc.sync.dma_start(
                            out=xs, in_=x[st * 128 : (st + 1) * 128, :]
                        )
                    ssl = slice(st * 128, (st + 1) * 128)
                    for cg in range(2):  # groups of 4 contraction tiles
                        tp = tp_pool.tile([128, 512], f32, tag="tp")
                        for c4 in range(4):
                            c = cg * 4 + c4
                            nc.tensor.transpose(
                                tp[:, c4 * 128 : (c4 + 1) * 128],
                                xs[:, c * 128 : (c + 1) * 128],
                                identity,
                            )
                        nc.vector.tensor_copy(
                            xt[:, cg * 4 : (cg + 1) * 4, ssl],
                            tp[:].rearrange("p (c s) -> p c s", c=4),
                        )
                    # V projection for this s-tile (natural layout).
                    vps = proj_pool.tile([128, DKL], f32, tag="proj")
                    for c in range(NC_):
                        nc.tensor.matmul(
                            vps[:],
                            lhsT=xt[:, c, ssl],
                            rhs=wv_sb[:, c, :],
                            start=(c == 0),
                            stop=(c == NC_ - 1),
                            skip_group_check=True,
                        )
                    nc.vector.tensor_copy(
                        vaug[:, st, :, 0:HD],
                        vps[:].rearrange("p (h d) -> p h d", h=NHL),
                    )
                    # After each 512-row span: Q^T / K^T blocks for that span.
                    if st % 4 == 3:
                        q2 = st // 4
                        sl = slice(q2 * 512, (q2 + 1) * 512)
                        for wsb, dst in ((wq_sb, qt), (wk_sb, kt)):
                            for j in range(2):
                                ps = proj_pool.tile([128, 512], f32, tag="proj")
                                for c in range(NC_):
                                    nc.tensor.matmul(
                                        ps[:],
                                        lhsT=wsb[:, c, j * 128 : (j + 1) * 128],
                                        rhs=xt[:, c, sl],
                                        start=(c == 0),
                                        stop=(c == NC_ - 1),
                                        skip_group_check=True,
                                    )
                                nc.vector.tensor_copy(dst[j][:, sl], ps[:])

            # ---------------- Phase C: attention + output ------------------
            with (
                tc.tile_pool(name="sc_ps", bufs=3, space="PSUM") as sc_pool,
                tc.tile_pool(name="mm4_ps", bufs=2, space="PSUM") as mm4_pool,
                tc.tile_pool(name="pt", bufs=6) as pt_pool,
                tc.tile_pool(name="cc", bufs=4) as cc_pool,
                tc.tile_pool(name="rb", bufs=3) as rb_pool,
                tc.tile_pool(name="yo", bufs=3) as yo_pool,
            ):

                def emit_norm(h, mm4ps, c_tiles):
                    """concat^T[h rows] = heads^T / rowsum (deferred)."""
                    pt_i = h // 2
                    hp = (h % 2) * HD
                    recip_f = rb_pool.tile([1, 512], f32, tag="recipf", name=f"rcf{h}")
                    nc.vector.reciprocal(recip_f[:], mm4ps[HD : HD + 1, :])
                    recip = rb_pool.tile([1, 512], f32r, tag="recip", name=f"rc{h}")
                    nc.vector.tensor_copy(recip[:], recip_f[:])
                    bc = sc_pool.tile([HD, 512], f32, tag="sc", name=f"bc{h}")
                    nc.tensor.matmul(
                        bc[:],
                        lhsT=ones_t[:],
                        rhs=recip[:],
                        start=True,
                        stop=True,
                        skip_group_check=True,
                    )
                    rbt = rb_pool.tile([HD, 512], f32, tag="rb", name=f"rb{h}")
                    nc.vector.tensor_copy(rbt[:], bc[:])
                    nc.vector.tensor_mul(
                        c_tiles[pt_i][hp : hp + HD, :], mm4ps[0:HD, :], rbt[:]
                    )

                for q in range(NQC):
                    qsl = slice(q * 512, (q + 1) * 512)
                    c_tiles = [
                        cc_pool.tile([128, 512], f32r, tag="cc", name=f"cc{q}_{i}")
                        for i in range(2)
                    ]
                    pending = []
                    for pi in range(2):  # head pairs (0,1) and (2,3)
                        h0, h1 = 2 * pi, 2 * pi + 1
                        mm4 = [
                            mm4_pool.tile(
                                [HD + 1, 512], f32, tag="mm4", name=f"mm4_{q}_{h}"
                            )
                            for h in (h0, h1)
                        ]

                        def emit_mm4(hi, pt_g, g0, glen, mm4=mm4, pi=pi):
                            for j in range(glen):
                                t = g0 + j
                                nc.tensor.matmul(
                                    mm4[hi][:],
                                    lhsT=vaug[:, t, 2 * pi + hi, :],
                                    rhs=pt_g[hi][:, j * 512 : (j + 1) * 512],
                                    start=(t == 0),
                                    stop=(t == NST - 1),
                                    skip_group_check=True,
                                )

                        prev = None
                        for gi, (g0, glen) in enumerate(GROUPS2):
                            sc = [
                                sc_pool.tile(
                                    [128, glen * 512],
                                    f32,
                                    tag="sc",
                                    name=f"sc{q}_{pi}_{gi}_{i}",
                                )
                                for i in range(2)
                            ]
                            for j in range(glen):
                                t = g0 + j
                                tsl = slice(t * 128, (t + 1) * 128)
                                osl = slice(j * 512, (j + 1) * 512)
                                for hi, hpp in ((0, 0), (1, HD)):
                                    nc.tensor.matmul(
                                        sc[hi][:, osl],
                                        lhsT=kt[pi][hpp : hpp + HD, tsl],
                                        rhs=qt[pi][hpp : hpp + HD, qsl],
                                        start=True,
                                        stop=True,
                                        skip_group_check=True,
                                    )
                            pt_g = [
                                pt_pool.tile(
                                    [128, glen * 512],
                                    f32r,
                                    tag="pt",
                                    name=f"pt{q}_{pi}_{gi}_{i}",
                                )
                                for i in range(2)
                            ]
                            for hi in range(2):
                                nc.scalar.activation(
                                    out=pt_g[hi][:],
                                    in_=sc[hi][:],
                                    func=EXP,
                                    scale=0.125,
                                )
                            if prev is not None:
                                for hi in range(2):
                                    emit_mm4(hi, *prev)
                            if gi == 1 and pending:
                                for h, m in pending:
                                    emit_norm(h, m, c_tiles)
                                pending = []
                            prev = (pt_g, g0, glen)
                        for hi in range(2):
                            emit_mm4(hi, *prev)
                        pending = [(h0, mm4[0]), (h1, mm4[1])]
                    for h, m in pending:
                        emit_norm(h, m, c_tiles)

                    # Output projection for this 512-row span.
                    for sub in range(4):
                        yo = yo_pool.tile([128, DM], f32, tag="yo")
                        for nh2 in range(2):
                            ps5 = mm4_pool.tile(
                                [128, 512], f32, tag="mm4", name=f"ps5_{q}_{sub}_{nh2}"
                            )
                            for c2 in range(2):
                                nc.tensor.matmul(
                                    ps5[:],
                                    lhsT=c_tiles[c2][:, sub * 128 : (sub + 1) * 128],
                                    rhs=w0_sb[:, c2, nh2 * 512 : (nh2 + 1) * 512],
                                    start=(c2 == 0),
                                    stop=(c2 == 1),
                                    skip_group_check=True,
                                )
                            nc.vector.tensor_copy(
                                yo[:, nh2 * 512 : (nh2 + 1) * 512], ps5[:]
                            )
                        r0 = q * 512 + sub * 128
                        nc.sync.dma_start(out=y[r0 : r0 + 128, :], in_=yo[:])

    nc.compile()
    return nc


def _get_nc():
    if "nc" not in _CACHE:
        _CACHE["nc"] = _build_nc()
    return _CACHE["nc"]


def kernel(inputs, Wq, Wk, Wv, W0):
    global LAST_EXEC_NS
    from concourse.bass_utils import run_bass_kernel_spmd

    inputs = np.asarray(inputs, dtype=np.float32)
    Wq = np.asarray(Wq, dtype=np.float32)
    Wk = np.asarray(Wk, dtype=np.float32)
    Wv = np.asarray(Wv, dtype=np.float32)
    W0 = np.asarray(W0, dtype=np.float32)

    trace = bool(os.environ.get("BASS_KERNEL_TRACE"))
    if trace:
        trace = _maybe_enable_trace()

    nc = _get_nc()
    in_maps = []
    for c in range(8):
        b, g = divmod(c, 4)
        sl = slice(DKL * g, DKL * (g + 1))
        in_maps.append(
            {
                "x": np.ascontiguousarray(inputs[b]),
                "wq": np.ascontiguousarray(Wq[:, sl]),
                "wk": np.ascontiguousarray(Wk[:, sl]),
                "wv": np.ascontiguousarray(Wv[:, sl]),
                "w0": np.ascontiguousarray(W0[sl, :]),
            }
        )
    res = run_bass_kernel_spmd(nc, in_maps, list(range(8)), trace=trace)
    LAST_EXEC_NS = res.exec_time_ns
    outs = [res.results[i]["y"] for i in range(8)]
    out = np.stack(
        [
            outs[0] + outs[1] + outs[2] + outs[3],
            outs[4] + outs[5] + outs[6] + outs[7],
        ]
    )
    return out.astype(np.float32)


# revision 11
# speedup vs baseline: 1.1251x; 1.0051x over previous
"""Trainium2 Bass kernel for 16-head dense multi-head attention.

Problem: B=2, S=2048, d_model=1024, 16 heads (head dim 64), fp32.
Sharding over 8 NeuronCores: core c -> batch b = c//4, head-group g = c%4
(4 heads, i.e. columns 256g:256(g+1) of Wq/Wk/Wv and rows 256g:256(g+1)
of W0). Each core computes a partial output projection for its batch;
the host sums the 4 partials per batch.

Per-core design (everything stays transposed so softmax denominators come
free):
  - X^T materialized in SBUF via PE transposes (fp32 exact), 4 blocks per
    PSUM bank so the PSUM->SBUF copy runs at [128,512] granularity.
  - Q^T, K^T projections with W tiles stationary, X^T moving.
  - scores^T for a head PAIR computed with row-group-disjoint matmuls
    (heads live in partitions 0:64 / 64:128 of the Q^T/K^T tiles), which
    the PE runs concurrently via its LDWEIGHTS reorder window.
  - exp via ACT directly out of PSUM over [128,1024] supertiles
    (scale=1/8 folded into the activation's free affine).
  - V augmented with a ones column -> P @ [V|1] gives both the head output
    and the softmax denominator in one PSUM accumulation (M=65).
  - reciprocal of the denominator broadcast across partitions with a K=1
    PE matmul; normalization fused into the PSUM->SBUF copy (DVE multiply)
    and emission-deferred so the PE never stalls on the DVE chain.
  - output projection accumulates into a [128,1024] PSUM supertile.
All big matmuls run in float32r (single-pass PE mode, ~1.5e-4 rel error).
"""

import os
import sys

import numpy as np

if "/opt/trn_rl_repo" not in sys.path:
    sys.path.insert(0, "/opt/trn_rl_repo")

S = 2048
DM = 1024
DKL = 256  # local q/k/v width (4 heads x 64)
HD = 64
NHL = 4  # heads per core
NST = S // 128  # 16 s-tiles
NC_ = DM // 128  # 8 contraction tiles
NQC = 4  # sq chunks of 512
GROUPS2 = [(g, 2) for g in range(0, 16, 2)]  # 8 groups of 2 sk-tiles

_CACHE = {}
LAST_EXEC_NS = None


def _maybe_enable_trace():
    try:
        import antenv

        p = "/opt/trn_rl_repo/antenv"
        if p not in antenv.__path__:
            antenv.__path__.append(p)
        from antenv.axon_hooks import (
            get_axon_ntff_profile_hook,
            set_axon_ntff_profile_hook,
        )

        if get_axon_ntff_profile_hook() is None:
            from trn_agent_boot.trn_boot import _ntff_profile_via_ctypes

            set_axon_ntff_profile_hook(
                _ntff_profile_via_ctypes("/opt/axon/libaxon_pjrt.so")
            )
        return True
    except Exception:
        return False


def _build_nc():
    import concourse.tile as tile
    from concourse import bacc, mybir
    from concourse.masks import make_identity

    f32 = mybir.dt.float32
    f32r = getattr(mybir.dt, os.environ.get("MM_DTYPE", "float32r"))
    EXP = mybir.ActivationFunctionType.Exp

    nc = bacc.Bacc("TRN2", target_bir_lowering=False, debug=False, num_devices=8)
    x = nc.dram_tensor("x", [S, DM], f32, kind="ExternalInput").ap()
    wq_d = nc.dram_tensor("wq", [DM, DKL], f32, kind="ExternalInput").ap()
    wk_d = nc.dram_tensor("wk", [DM, DKL], f32, kind="ExternalInput").ap()
    wv_d = nc.dram_tensor("wv", [DM, DKL], f32, kind="ExternalInput").ap()
    w0_d = nc.dram_tensor("w0", [DKL, DM], f32, kind="ExternalInput").ap()
    y = nc.dram_tensor("y", [S, DM], f32, kind="ExternalOutput").ap()

    with tile.TileContext(nc) as tc:
        with (
            tc.tile_pool(name="const", bufs=1) as const_pool,
            tc.tile_pool(name="wpool", bufs=1) as w_pool,
            tc.tile_pool(name="qkv", bufs=1) as qkv_pool,
        ):
            identity = const_pool.tile([128, 128], f32)
            make_identity(nc, identity)
            ones_f = const_pool.tile([1, HD], f32)
            nc.vector.memset(ones_f, 1.0)
            ones_t = const_pool.tile([1, HD], f32r)
            nc.vector.tensor_copy(ones_t[:], ones_f[:])
            ones_col = const_pool.tile([128, NST, NHL, 1], f32)
            nc.vector.memset(ones_col, 1.0)

            wq_sb = w_pool.tile([128, NC_, DKL], f32r)
            wk_sb = w_pool.tile([128, NC_, DKL], f32r)
            wv_sb = w_pool.tile([128, NC_, DKL], f32r)
            w0_sb = w_pool.tile([128, DKL // 128, DM], f32r)

            # Persistent transposed activations / projections.
            qt = [qkv_pool.tile([128, S], f32r, name=f"qt{i}") for i in range(2)]
            kt = [qkv_pool.tile([128, S], f32r, name=f"kt{i}") for i in range(2)]
            vaug = qkv_pool.tile([128, NST, NHL, HD + 1], f32r)
            nc.vector.tensor_copy(vaug[:, :, :, HD : HD + 1], ones_col[:])

            # ---------------- Phase B: X^T + Q/K/V projections -------------
            with (
                tc.tile_pool(name="xs", bufs=10) as xs_pool,
                tc.tile_pool(name="xt", bufs=1) as xt_pool,
                tc.tile_pool(name="tp_ps", bufs=4, space="PSUM") as tp_pool,
                tc.tile_pool(name="proj_ps", bufs=3, space="PSUM") as proj_pool,
            ):
                xt = xt_pool.tile([128, NC_, S], f32r)
                xs_tiles = {}
                for st in range(2):
                    xs_tiles[st] = xs_pool.tile([128, DM], f32, tag="xs", name=f"xs{st}")
                    nc.sync.dma_start(
                        out=xs_tiles[st], in_=x[st * 128 : (st + 1) * 128, :]
                    )
                for wi, (dst2, src2) in enumerate(
                    (
                        (wq_sb, wq_d),
                        (wk_sb, wk_d),
                        (wv_sb, wv_d),
                        (w0_sb, w0_d),
                    )
                ):
                    if f32r == mybir.dt.float32r:
                        nc.sync.dma_start(
                            out=dst2,
                            in_=src2.bitcast(f32r).rearrange(
                                "(t p) c -> p t c", p=128
                            ),
                        )
                    else:
                        wst = xs_pool.tile(
                            [128, dst2.shape[1], dst2.shape[2]],
                            f32,
                            tag="xs",
                            name=f"wst{wi}",
                        )
                        nc.sync.dma_start(
                            out=wst,
                            in_=src2.rearrange("(t p) c -> p t c", p=128),
                        )
                        nc.vector.tensor_copy(dst2[:], wst[:])
                for st in range(NST):
                    if st in xs_tiles:
                        xs = xs_tiles[st]
                    else:
                        xs = xs_pool.tile([128, DM], f32, tag="xs", name=f"xs{st}")
                        nc.sync.dma_start(
                            out=xs, in_=x[st * 128 : (st + 1) * 128, :]
                        )
                    ssl = slice(st * 128, (st + 1) * 128)
                    for cg in range(2):  # groups of 4 contraction tiles
                        tp = tp_pool.tile([128, 512], f32, tag="tp")
                        for c4 in range(4):
                            c = cg * 4 + c4
                            nc.tensor.transpose(
                                tp[:, c4 * 128 : (c4 + 1) * 128],
                                xs[:, c * 128 : (c + 1) * 128],
                                identity,
                            )
                        nc.vector.tensor_copy(
                            xt[:, cg * 4 : (cg + 1) * 4, ssl],
                            tp[:].rearrange("p (c s) -> p c s", c=4),
                        )
                    # V projection for this s-tile (natural layout).
                    vps = proj_pool.tile([128, DKL], f32, tag="proj")
                    for c in range(NC_):
                        nc.tensor.matmul(
                            vps[:],
                            lhsT=xt[:, c, ssl],
                            rhs=wv_sb[:, c, :],
                            start=(c == 0),
                            stop=(c == NC_ - 1),
                            skip_group_check=True,
                        )
                    nc.vector.tensor_copy(
                        vaug[:, st, :, 0:HD],
                        vps[:].rearrange("p (h d) -> p h d", h=NHL),
                    )
                    # After each 512-row span: Q^T / K^T blocks for that span.
                    if st % 4 == 3:
                        q2 = st // 4
                        sl = slice(q2 * 512, (q2 + 1) * 512)
                        for wsb, dst in ((wq_sb, qt), (wk_sb, kt)):
                            for j in range(2):
                                ps = proj_pool.tile([128, 512], f32, tag="proj")
                                for c in range(NC_):
                                    nc.tensor.matmul(
                                        ps[:],
                                        lhsT=wsb[:, c, j * 128 : (j + 1) * 128],
                                        rhs=xt[:, c, sl],
                                        start=(c == 0),
                                        stop=(c == NC_ - 1),
                                        skip_group_check=True,
                                    )
                                nc.vector.tensor_copy(dst[j][:, sl], ps[:])

            # ---------------- Phase C: attention + output ------------------
            with (
                tc.tile_pool(name="sc_ps", bufs=3, space="PSUM") as sc_pool,
                tc.tile_pool(name="mm4_ps", bufs=2, space="PSUM") as mm4_pool,
                tc.tile_pool(name="pt", bufs=8) as pt_pool,
                tc.tile_pool(name="cc", bufs=6) as cc_pool,
                tc.tile_pool(name="rb", bufs=6) as rb_pool,
                tc.tile_pool(name="yo", bufs=4) as yo_pool,
            ):

                def emit_norm(h, mm4ps, c_tiles):
                    """concat^T[h rows] = heads^T / rowsum (deferred)."""
                    pt_i = h // 2
                    hp = (h % 2) * HD
                    recip_f = rb_pool.tile([1, 512], f32, tag="recipf", name=f"rcf{h}")
                    nc.vector.reciprocal(recip_f[:], mm4ps[HD : HD + 1, :])
                    recip = rb_pool.tile([1, 512], f32r, tag="recip", name=f"rc{h}")
                    nc.vector.tensor_copy(recip[:], recip_f[:])
                    bc = sc_pool.tile([HD, 512], f32, tag="sc", name=f"bc{h}")
                    nc.tensor.matmul(
                        bc[:],
                        lhsT=ones_t[:],
                        rhs=recip[:],
                        start=True,
                        stop=True,
                        skip_group_check=True,
                    )
                    rbt = rb_pool.tile([HD, 512], f32, tag="rb", name=f"rb{h}")
                    nc.vector.tensor_copy(rbt[:], bc[:])
                    nc.vector.tensor_mul(
                        c_tiles[pt_i][hp : hp + HD, :], mm4ps[0:HD, :], rbt[:]
                    )

                for q in range(NQC):
                    qsl = slice(q * 512, (q + 1) * 512)
                    c_tiles = [
                        cc_pool.tile([128, 512], f32r, tag="cc", name=f"cc{q}_{i}")
                        for i in range(2)
                    ]
                    pending = []
                    for pi in range(2):  # head pairs (0,1) and (2,3)
                        h0, h1 = 2 * pi, 2 * pi + 1
                        mm4 = [
                            mm4_pool.tile(
                                [HD + 1, 512], f32, tag="mm4", name=f"mm4_{q}_{h}"
                            )
                            for h in (h0, h1)
                        ]

                        def emit_mm4(hi, pt_g, g0, glen, mm4=mm4, pi=pi):
                            for j in range(glen):
                                t = g0 + j
                                nc.tensor.matmul(
                                    mm4[hi][:],
                                    lhsT=vaug[:, t, 2 * pi + hi, :],
                                    rhs=pt_g[hi][:, j * 512 : (j + 1) * 512],
                                    start=(t == 0),
                                    stop=(t == NST - 1),
                                    skip_group_check=True,
                                )

                        prev = None
                        for gi, (g0, glen) in enumerate(GROUPS2):
                            sc = [
                                sc_pool.tile(
                                    [128, glen * 512],
                                    f32,
                                    tag="sc",
                                    name=f"sc{q}_{pi}_{gi}_{i}",
                                )
                                for i in range(2)
                            ]
                            for j in range(glen):
                                t = g0 + j
                                tsl = slice(t * 128, (t + 1) * 128)
                                osl = slice(j * 512, (j + 1) * 512)
                                for hi, hpp in ((0, 0), (1, HD)):
                                    nc.tensor.matmul(
                                        sc[hi][:, osl],
                                        lhsT=kt[pi][hpp : hpp + HD, tsl],
                                        rhs=qt[pi][hpp : hpp + HD, qsl],
                                        start=True,
                                        stop=True,
                                        skip_group_check=True,
                                    )
                            pt_g = [
                                pt_pool.tile(
                                    [128, glen * 512],
                                    f32r,
                                    tag="pt",
                                    name=f"pt{q}_{pi}_{gi}_{i}",
                                )
                                for i in range(2)
                            ]
                            for hi in range(2):
                                nc.scalar.activation(
                                    out=pt_g[hi][:],
                                    in_=sc[hi][:],
                                    func=EXP,
                                    scale=0.125,
                                )
                            if prev is not None:
                                for hi in range(2):
                                    emit_mm4(hi, *prev)
                            if gi == 1 and pending:
                                for h, m in pending:
                                    emit_norm(h, m, c_tiles)
                                pending = []
                            prev = (pt_g, g0, glen)
                        for hi in range(2):
                            emit_mm4(hi, *prev)
                        pending = [(h0, mm4[0]), (h1, mm4[1])]
                    for h, m in pending:
                        emit_norm(h, m, c_tiles)

                    # Output projection for this 512-row span.
                    for sub in range(4):
                        yo = yo_pool.tile([128, DM], f32, tag="yo")
                        for nh2 in range(2):
                            ps5 = mm4_pool.tile(
                                [128, 512], f32, tag="mm4", name=f"ps5_{q}_{sub}_{nh2}"
                            )
                            for c2 in range(2):
                                nc.tensor.matmul(
                                    ps5[:],
                                    lhsT=c_tiles[c2][:, sub * 128 : (sub + 1) * 128],
                                    rhs=w0_sb[:, c2, nh2 * 512 : (nh2 + 1) * 512],
                                    start=(c2 == 0),
                                    stop=(c2 == 1),
                                    skip_group_check=True,
                                )
                            nc.vector.tensor_copy(
                                yo[:, nh2 * 512 : (nh2 + 1) * 512], ps5[:]
                            )
                        r0 = q * 512 + sub * 128
                        nc.sync.dma_start(out=y[r0 : r0 + 128, :], in_=yo[:])

    nc.compile()
    return nc


def _get_nc():
    if "nc" not in _CACHE:
        _CACHE["nc"] = _build_nc()
    return _CACHE["nc"]


def kernel(inputs, Wq, Wk, Wv, W0):
    global LAST_EXEC_NS
    from concourse.bass_utils import run_bass_kernel_spmd

    inputs = np.asarray(inputs, dtype=np.float32)
    Wq = np.asarray(Wq, dtype=np.float32)
    Wk = np.asarray(Wk, dtype=np.float32)
    Wv = np.asarray(Wv, dtype=np.float32)
    W0 = np.asarray(W0, dtype=np.float32)

    trace = bool(os.environ.get("BASS_KERNEL_TRACE"))
    if trace:
        trace = _maybe_enable_trace()

    nc = _get_nc()
    in_maps = []
    for c in range(8):
        b, g = divmod(c, 4)
        sl = slice(DKL * g, DKL * (g + 1))
        in_maps.append(
            {
                "x": np.ascontiguousarray(inputs[b]),
                "wq": np.ascontiguousarray(Wq[:, sl]),
                "wk": np.ascontiguousarray(Wk[:, sl]),
                "wv": np.ascontiguousarray(Wv[:, sl]),
                "w0": np.ascontiguousarray(W0[sl, :]),
            }
        )
    res = run_bass_kernel_spmd(nc, in_maps, list(range(8)), trace=trace)
    LAST_EXEC_NS = res.exec_time_ns
    outs = [res.results[i]["y"] for i in range(8)]
    out = np.stack(
        [
            outs[0] + outs[1] + outs[2] + outs[3],
            outs[4] + outs[5] + outs[6] + outs[7],
        ]
    )
    return out.astype(np.float32)


# revision 13
# speedup vs baseline: 1.1269x; 1.0016x over previous
"""Trainium2 Bass kernel for 16-head dense multi-head attention.

Problem: B=2, S=2048, d_model=1024, 16 heads (head dim 64), fp32.
Sharding over 8 NeuronCores: core c -> batch b = c//4, head-group g = c%4
(4 heads, i.e. columns 256g:256(g+1) of Wq/Wk/Wv and rows 256g:256(g+1)
of W0). Each core computes a partial output projection for its batch;
the host sums the 4 partials per batch.

Per-core design (everything stays transposed so softmax denominators come
free):
  - X^T materialized in SBUF via PE transposes (fp32 exact), 4 blocks per
    PSUM bank so the PSUM->SBUF copy runs at [128,512] granularity.
  - Q^T, K^T projections with W tiles stationary, X^T moving.
  - scores^T for a head PAIR computed with row-group-disjoint matmuls
    (heads live in partitions 0:64 / 64:128 of the Q^T/K^T tiles), which
    the PE runs concurrently via its LDWEIGHTS reorder window.
  - exp via ACT directly out of PSUM over [128,1024] supertiles
    (scale=1/8 folded into the activation's free affine).
  - V augmented with a ones column -> P @ [V|1] gives both the head output
    and the softmax denominator in one PSUM accumulation (M=65).
  - reciprocal of the denominator broadcast across partitions with a K=1
    PE matmul; normalization fused into the PSUM->SBUF copy (DVE multiply)
    and emission-deferred so the PE never stalls on the DVE chain.
  - output projection shares the mm4 pool's PSUM bank slots; scores get
    three [128,1024] supertile slots (6 banks) of lookahead, which keeps
    the PE dense enough to limit HAM clock-governor throttling.
All big matmuls run in float32r (single-pass PE mode, ~1.5e-4 rel error).
"""

import os
import sys

import numpy as np

if "/opt/trn_rl_repo" not in sys.path:
    sys.path.insert(0, "/opt/trn_rl_repo")

S = 2048
DM = 1024
DKL = 256  # local q/k/v width (4 heads x 64)
HD = 64
NHL = 4  # heads per core
NST = S // 128  # 16 s-tiles
NC_ = DM // 128  # 8 contraction tiles
NQC = 4  # sq chunks of 512
GROUPS2 = [(g, 2) for g in range(0, 16, 2)]  # 8 groups of 2 sk-tiles

_CACHE = {}
LAST_EXEC_NS = None


def _maybe_enable_trace():
    try:
        import antenv

        p = "/opt/trn_rl_repo/antenv"
        if p not in antenv.__path__:
            antenv.__path__.append(p)
        from antenv.axon_hooks import (
            get_axon_ntff_profile_hook,
            set_axon_ntff_profile_hook,
        )

        if get_axon_ntff_profile_hook() is None:
            from trn_agent_boot.trn_boot import _ntff_profile_via_ctypes

            set_axon_ntff_profile_hook(
                _ntff_profile_via_ctypes("/opt/axon/libaxon_pjrt.so")
            )
        return True
    except Exception:
        return False


def _build_nc():
    import concourse.tile as tile
    from concourse import bacc, mybir
    from concourse.masks import make_identity

    f32 = mybir.dt.float32
    f32r = getattr(mybir.dt, os.environ.get("MM_DTYPE", "float32r"))
    EXP = mybir.ActivationFunctionType.Exp

    nc = bacc.Bacc("TRN2", target_bir_lowering=False, debug=False, num_devices=8)
    x = nc.dram_tensor("x", [S, DM], f32, kind="ExternalInput").ap()
    wq_d = nc.dram_tensor("wq", [DM, DKL], f32, kind="ExternalInput").ap()
    wk_d = nc.dram_tensor("wk", [DM, DKL], f32, kind="ExternalInput").ap()
    wv_d = nc.dram_tensor("wv", [DM, DKL], f32, kind="ExternalInput").ap()
    w0_d = nc.dram_tensor("w0", [DKL, DM], f32, kind="ExternalInput").ap()
    y = nc.dram_tensor("y", [S, DM], f32, kind="ExternalOutput").ap()

    with tile.TileContext(nc) as tc:
        with (
            tc.tile_pool(name="const", bufs=1) as const_pool,
            tc.tile_pool(name="wpool", bufs=1) as w_pool,
            tc.tile_pool(name="qkv", bufs=1) as qkv_pool,
        ):
            identity = const_pool.tile([128, 128], f32)
            make_identity(nc, identity)
            ones_f = const_pool.tile([1, HD], f32)
            nc.vector.memset(ones_f, 1.0)
            ones_t = const_pool.tile([1, HD], f32r)
            nc.vector.tensor_copy(ones_t[:], ones_f[:])
            ones_col = const_pool.tile([128, NST, NHL, 1], f32)
            nc.vector.memset(ones_col, 1.0)

            wq_sb = w_pool.tile([128, NC_, DKL], f32r)
            wk_sb = w_pool.tile([128, NC_, DKL], f32r)
            wv_sb = w_pool.tile([128, NC_, DKL], f32r)
            w0_sb = w_pool.tile([128, DKL // 128, DM], f32r)

            # Persistent transposed activations / projections.
            qt = [qkv_pool.tile([128, S], f32r, name=f"qt{i}") for i in range(2)]
            kt = [qkv_pool.tile([128, S], f32r, name=f"kt{i}") for i in range(2)]
            vaug = qkv_pool.tile([128, NST, NHL, HD + 1], f32r)
            nc.vector.tensor_copy(vaug[:, :, :, HD : HD + 1], ones_col[:])

            # ---------------- Phase B: X^T + Q/K/V projections -------------
            with (
                tc.tile_pool(name="xs", bufs=10) as xs_pool,
                tc.tile_pool(name="xt", bufs=1) as xt_pool,
                tc.tile_pool(name="tp_ps", bufs=4, space="PSUM") as tp_pool,
                tc.tile_pool(name="proj_ps", bufs=4, space="PSUM") as proj_pool,
            ):
                xt = xt_pool.tile([128, NC_, S], f32r)
                xs_tiles = {}
                for st in range(2):
                    xs_tiles[st] = xs_pool.tile([128, DM], f32, tag="xs", name=f"xs{st}")
                    nc.sync.dma_start(
                        out=xs_tiles[st], in_=x[st * 128 : (st + 1) * 128, :]
                    )
                for wi, (dst2, src2) in enumerate(
                    (
                        (wq_sb, wq_d),
                        (wk_sb, wk_d),
                        (wv_sb, wv_d),
                        (w0_sb, w0_d),
                    )
                ):
                    if f32r == mybir.dt.float32r:
                        nc.sync.dma_start(
                            out=dst2,
                            in_=src2.bitcast(f32r).rearrange(
                                "(t p) c -> p t c", p=128
                            ),
                        )
                    else:
                        wst = xs_pool.tile(
                            [128, dst2.shape[1], dst2.shape[2]],
                            f32,
                            tag="xs",
                            name=f"wst{wi}",
                        )
                        nc.sync.dma_start(
                            out=wst,
                            in_=src2.rearrange("(t p) c -> p t c", p=128),
                        )
                        nc.vector.tensor_copy(dst2[:], wst[:])
                for st in range(NST):
                    if st in xs_tiles:
                        xs = xs_tiles[st]
                    else:
                        xs = xs_pool.tile([128, DM], f32, tag="xs", name=f"xs{st}")
                        nc.sync.dma_start(
                            out=xs, in_=x[st * 128 : (st + 1) * 128, :]
                        )
                    ssl = slice(st * 128, (st + 1) * 128)
                    for cg in range(2):  # groups of 4 contraction tiles
                        tp = tp_pool.tile([128, 512], f32, tag="tp")
                        for c4 in range(4):
                            c = cg * 4 + c4
                            nc.tensor.transpose(
                                tp[:, c4 * 128 : (c4 + 1) * 128],
                                xs[:, c * 128 : (c + 1) * 128],
                                identity,
                            )
                        nc.vector.tensor_copy(
                            xt[:, cg * 4 : (cg + 1) * 4, ssl],
                            tp[:].rearrange("p (c s) -> p c s", c=4),
                        )
                    # V projection for this s-tile (natural layout).
                    vps = proj_pool.tile([128, DKL], f32, tag="proj")
                    for c in range(NC_):
                        nc.tensor.matmul(
                            vps[:],
                            lhsT=xt[:, c, ssl],
                            rhs=wv_sb[:, c, :],
                            start=(c == 0),
                            stop=(c == NC_ - 1),
                            skip_group_check=True,
                        )
                    nc.vector.tensor_copy(
                        vaug[:, st, :, 0:HD],
                        vps[:].rearrange("p (h d) -> p h d", h=NHL),
                    )
                    # After each 512-row span: Q^T / K^T blocks for that span.
                    if st % 4 == 3:
                        q2 = st // 4
                        sl = slice(q2 * 512, (q2 + 1) * 512)
                        for wsb, dst in ((wq_sb, qt), (wk_sb, kt)):
                            for j in range(2):
                                ps = proj_pool.tile([128, 512], f32, tag="proj")
                                for c in range(NC_):
                                    nc.tensor.matmul(
                                        ps[:],
                                        lhsT=wsb[:, c, j * 128 : (j + 1) * 128],
                                        rhs=xt[:, c, sl],
                                        start=(c == 0),
                                        stop=(c == NC_ - 1),
                                        skip_group_check=True,
                                    )
                                nc.vector.tensor_copy(dst[j][:, sl], ps[:])

            # ---------------- Phase C: attention + output ------------------
            with (
                tc.tile_pool(name="sc_ps", bufs=3, space="PSUM") as sc_pool,
                tc.tile_pool(name="mm4_ps", bufs=2, space="PSUM") as mm4_pool,
                tc.tile_pool(name="pt", bufs=8) as pt_pool,
                tc.tile_pool(name="cc", bufs=6) as cc_pool,
                tc.tile_pool(name="rb", bufs=6) as rb_pool,
                tc.tile_pool(name="yo", bufs=4) as yo_pool,
            ):

                def emit_norm(h, mm4ps, c_tiles):
                    """concat^T[h rows] = heads^T / rowsum (deferred)."""
                    pt_i = h // 2
                    hp = (h % 2) * HD
                    recip_f = rb_pool.tile([1, 512], f32, tag="recipf", name=f"rcf{h}")
                    nc.vector.reciprocal(recip_f[:], mm4ps[HD : HD + 1, :])
                    recip = rb_pool.tile([1, 512], f32r, tag="recip", name=f"rc{h}")
                    nc.vector.tensor_copy(recip[:], recip_f[:])
                    bc = sc_pool.tile([HD, 512], f32, tag="sc", name=f"bc{h}")
                    nc.tensor.matmul(
                        bc[:],
                        lhsT=ones_t[:],
                        rhs=recip[:],
                        start=True,
                        stop=True,
                        skip_group_check=True,
                    )
                    rbt = rb_pool.tile([HD, 512], f32, tag="rb", name=f"rb{h}")
                    nc.vector.tensor_copy(rbt[:], bc[:])
                    nc.vector.tensor_mul(
                        c_tiles[pt_i][hp : hp + HD, :], mm4ps[0:HD, :], rbt[:]
                    )

                for q in range(NQC):
                    qsl = slice(q * 512, (q + 1) * 512)
                    c_tiles = [
                        cc_pool.tile([128, 512], f32r, tag="cc", name=f"cc{q}_{i}")
                        for i in range(2)
                    ]
                    pending = []
                    for pi in range(2):  # head pairs (0,1) and (2,3)
                        h0, h1 = 2 * pi, 2 * pi + 1
                        mm4 = [
                            mm4_pool.tile(
                                [HD + 1, 512], f32, tag="mm4", name=f"mm4_{q}_{h}"
                            )
                            for h in (h0, h1)
                        ]

                        def emit_mm4(hi, pt_g, g0, glen, mm4=mm4, pi=pi):
                            for j in range(glen):
                                t = g0 + j
                                nc.tensor.matmul(
                                    mm4[hi][:],
                                    lhsT=vaug[:, t, 2 * pi + hi, :],
                                    rhs=pt_g[hi][:, j * 512 : (j + 1) * 512],
                                    start=(t == 0),
                                    stop=(t == NST - 1),
                                    skip_group_check=True,
                                )

                        prev = None
                        for gi, (g0, glen) in enumerate(GROUPS2):
                            sc = [
                                sc_pool.tile(
                                    [128, glen * 512],
                                    f32,
                                    tag="sc",
                                    name=f"sc{q}_{pi}_{gi}_{i}",
                                )
                                for i in range(2)
                            ]
                            for j in range(glen):
                                t = g0 + j
                                tsl = slice(t * 128, (t + 1) * 128)
                                osl = slice(j * 512, (j + 1) * 512)
                                for hi, hpp in ((0, 0), (1, HD)):
                                    nc.tensor.matmul(
                                        sc[hi][:, osl],
                                        lhsT=kt[pi][hpp : hpp + HD, tsl],
                                        rhs=qt[pi][hpp : hpp + HD, qsl],
                                        start=True,
                                        stop=True,
                                        skip_group_check=True,
                                    )
                            pt_g = [
                                pt_pool.tile(
                                    [128, glen * 512],
                                    f32r,
                                    tag="pt",
                                    name=f"pt{q}_{pi}_{gi}_{i}",
                                )
                                for i in range(2)
                            ]
                            for hi in range(2):
                                nc.scalar.activation(
                                    out=pt_g[hi][:],
                                    in_=sc[hi][:],
                                    func=EXP,
                                    scale=0.125,
                                )
                            if prev is not None:
                                for hi in range(2):
                                    emit_mm4(hi, *prev)
                            if gi == 1 and pending:
                                for h, m in pending:
                                    emit_norm(h, m, c_tiles)
                                pending = []
                            prev = (pt_g, g0, glen)
                        for hi in range(2):
                            emit_mm4(hi, *prev)
                        pending = [(h0, mm4[0]), (h1, mm4[1])]
                    for h, m in pending:
                        emit_norm(h, m, c_tiles)

                    # Output projection for this 512-row span.
                    for sub in range(4):
                        yo = yo_pool.tile([128, DM], f32, tag="yo")
                        for nh2 in range(2):
                            ps5 = mm4_pool.tile(
                                [128, 512], f32, tag="mm4", name=f"ps5_{q}_{sub}_{nh2}"
                            )
                            for c2 in range(2):
                                nc.tensor.matmul(
                                    ps5[:],
                                    lhsT=c_tiles[c2][:, sub * 128 : (sub + 1) * 128],
                                    rhs=w0_sb[:, c2, nh2 * 512 : (nh2 + 1) * 512],
                                    start=(c2 == 0),
                                    stop=(c2 == 1),
                                    skip_group_check=True,
                                )
                            nc.vector.tensor_copy(
                                yo[:, nh2 * 512 : (nh2 + 1) * 512], ps5[:]
                            )
                        r0 = q * 512 + sub * 128
                        nc.sync.dma_start(out=y[r0 : r0 + 128, :], in_=yo[:])

    nc.compile()
    return nc


def _get_nc():
    if "nc" not in _CACHE:
        _CACHE["nc"] = _build_nc()
    return _CACHE["nc"]


def kernel(inputs, Wq, Wk, Wv, W0):
    global LAST_EXEC_NS
    from concourse.bass_utils import run_bass_kernel_spmd

    inputs = np.asarray(inputs, dtype=np.float32)
    Wq = np.asarray(Wq, dtype=np.float32)
    Wk = np.asarray(Wk, dtype=np.float32)
    Wv = np.asarray(Wv, dtype=np.float32)
    W0 = np.asarray(W0, dtype=np.float32)

    trace = bool(os.environ.get("BASS_KERNEL_TRACE"))
    if trace:
        trace = _maybe_enable_trace()

    nc = _get_nc()
    in_maps = []
    for c in range(8):
        b, g = divmod(c, 4)
        sl = slice(DKL * g, DKL * (g + 1))
        in_maps.append(
            {
                "x": np.ascontiguousarray(inputs[b]),
                "wq": np.ascontiguousarray(Wq[:, sl]),
                "wk": np.ascontiguousarray(Wk[:, sl]),
                "wv": np.ascontiguousarray(Wv[:, sl]),
                "w0": np.ascontiguousarray(W0[sl, :]),
            }
        )
    res = run_bass_kernel_spmd(nc, in_maps, list(range(8)), trace=trace)
    LAST_EXEC_NS = res.exec_time_ns
    outs = [res.results[i]["y"] for i in range(8)]
    out = np.stack(
        [
            outs[0] + outs[1] + outs[2] + outs[3],
            outs[4] + outs[5] + outs[6] + outs[7],
        ]
    )
    return out.astype(np.float32)


# revision 14
# speedup vs baseline: 1.1334x; 1.0058x over previous
"""Trainium2 Bass kernel for 16-head dense multi-head attention.

Problem: B=2, S=2048, d_model=1024, 16 heads (head dim 64), fp32.
Sharding over 8 NeuronCores: core c -> batch b = c//4, head-group g = c%4
(4 heads, i.e. columns 256g:256(g+1) of Wq/Wk/Wv and rows 256g:256(g+1)
of W0). Each core computes a partial output projection for its batch;
the host sums the 4 partials per batch.

Per-core design (everything stays transposed so softmax denominators come
free):
  - X^T materialized in SBUF via PE transposes (fp32 exact), 4 blocks per
    PSUM bank so the PSUM->SBUF copy runs at [128,512] granularity.
  - Q^T, K^T projections with W tiles stationary, X^T moving.
  - scores^T for a head PAIR computed with row-group-disjoint matmuls
    (heads live in partitions 0:64 / 64:128 of the Q^T/K^T tiles), which
    the PE runs concurrently via its LDWEIGHTS reorder window.
  - exp via ACT directly out of PSUM over [128,1024] supertiles
    (scale=1/8 folded into the activation's free affine).
  - V augmented with a ones column -> P @ [V|1] gives both the head output
    and the softmax denominator in one PSUM accumulation (M=65).
  - reciprocal of the denominator broadcast across partitions with a K=1
    PE matmul; normalization fused into the PSUM->SBUF copy (DVE multiply)
    and emission-deferred so the PE never stalls on the DVE chain.
  - output projection shares the mm4 pool's PSUM bank slots; scores get
    three [128,1024] supertile slots (6 banks) of lookahead, which keeps
    the PE dense enough to limit HAM clock-governor throttling.
All big matmuls run in float32r (single-pass PE mode, ~1.5e-4 rel error).
"""

import os
import sys

import numpy as np

if "/opt/trn_rl_repo" not in sys.path:
    sys.path.insert(0, "/opt/trn_rl_repo")

S = 2048
DM = 1024
DKL = 256  # local q/k/v width (4 heads x 64)
HD = 64
NHL = 4  # heads per core
NST = S // 128  # 16 s-tiles
NC_ = DM // 128  # 8 contraction tiles
NQC = 4  # sq chunks of 512
GROUPS2 = [(g, 2) for g in range(0, 16, 2)]  # 8 groups of 2 sk-tiles

_CACHE = {}
LAST_EXEC_NS = None


def _maybe_enable_trace():
    try:
        import antenv

        p = "/opt/trn_rl_repo/antenv"
        if p not in antenv.__path__:
            antenv.__path__.append(p)
        from antenv.axon_hooks import (
            get_axon_ntff_profile_hook,
            set_axon_ntff_profile_hook,
        )

        if get_axon_ntff_profile_hook() is None:
            from trn_agent_boot.trn_boot import _ntff_profile_via_ctypes

            set_axon_ntff_profile_hook(
                _ntff_profile_via_ctypes("/opt/axon/libaxon_pjrt.so")
            )
        return True
    except Exception:
        return False


def _build_nc():
    import concourse.tile as tile
    from concourse import bacc, mybir
    from concourse.masks import make_identity

    f32 = mybir.dt.float32
    f32r = getattr(mybir.dt, os.environ.get("MM_DTYPE", "float32r"))
    EXP = mybir.ActivationFunctionType.Exp

    nc = bacc.Bacc("TRN2", target_bir_lowering=False, debug=False, num_devices=8)
    x = nc.dram_tensor("x", [S, DM], f32, kind="ExternalInput").ap()
    wq_d = nc.dram_tensor("wq", [DM, DKL], f32, kind="ExternalInput").ap()
    wk_d = nc.dram_tensor("wk", [DM, DKL], f32, kind="ExternalInput").ap()
    wv_d = nc.dram_tensor("wv", [DM, DKL], f32, kind="ExternalInput").ap()
    w0_d = nc.dram_tensor("w0", [DKL, DM], f32, kind="ExternalInput").ap()
    y = nc.dram_tensor("y", [S, DM], f32, kind="ExternalOutput").ap()

    with tile.TileContext(nc) as tc:
        with (
            tc.tile_pool(name="const", bufs=1) as const_pool,
            tc.tile_pool(name="wpool", bufs=1) as w_pool,
            tc.tile_pool(name="qkv", bufs=1) as qkv_pool,
        ):
            identity = const_pool.tile([128, 128], f32)
            make_identity(nc, identity)
            ones_f = const_pool.tile([1, HD], f32)
            nc.vector.memset(ones_f, 1.0)
            ones_t = const_pool.tile([1, HD], f32r)
            nc.vector.tensor_copy(ones_t[:], ones_f[:])
            ones_col = const_pool.tile([128, NST, NHL, 1], f32)
            nc.vector.memset(ones_col, 1.0)

            wq_sb = w_pool.tile([128, NC_, DKL], f32r)
            wk_sb = w_pool.tile([128, NC_, DKL], f32r)
            wv_sb = w_pool.tile([128, NC_, DKL], f32r)
            w0_sb = w_pool.tile([128, DKL // 128, DM], f32r)

            # Persistent transposed activations / projections.
            qt = [qkv_pool.tile([128, S], f32r, name=f"qt{i}") for i in range(2)]
            kt = [qkv_pool.tile([128, S], f32r, name=f"kt{i}") for i in range(2)]
            vaug = qkv_pool.tile([128, NST, NHL, HD + 1], f32r)
            nc.vector.tensor_copy(vaug[:, :, :, HD : HD + 1], ones_col[:])

            # ---------------- Phase B: X^T + Q/K/V projections -------------
            with (
                tc.tile_pool(name="xs", bufs=10) as xs_pool,
                tc.tile_pool(name="xt", bufs=1) as xt_pool,
                tc.tile_pool(name="tp_ps", bufs=4, space="PSUM") as tp_pool,
                tc.tile_pool(name="proj_ps", bufs=4, space="PSUM") as proj_pool,
            ):
                xt = xt_pool.tile([128, NC_, S], f32r)
                xs_tiles = {}
                for st in range(2):
                    xs_tiles[st] = xs_pool.tile([128, DM], f32, tag="xs", name=f"xs{st}")
                    nc.sync.dma_start(
                        out=xs_tiles[st], in_=x[st * 128 : (st + 1) * 128, :]
                    )
                for wi, (dst2, src2) in enumerate(
                    (
                        (wq_sb, wq_d),
                        (wk_sb, wk_d),
                        (wv_sb, wv_d),
                        (w0_sb, w0_d),
                    )
                ):
                    if f32r == mybir.dt.float32r:
                        nc.sync.dma_start(
                            out=dst2,
                            in_=src2.bitcast(f32r).rearrange(
                                "(t p) c -> p t c", p=128
                            ),
                        )
                    else:
                        wst = xs_pool.tile(
                            [128, dst2.shape[1], dst2.shape[2]],
                            f32,
                            tag="xs",
                            name=f"wst{wi}",
                        )
                        nc.sync.dma_start(
                            out=wst,
                            in_=src2.rearrange("(t p) c -> p t c", p=128),
                        )
                        nc.vector.tensor_copy(dst2[:], wst[:])
                for st in range(NST):
                    if st in xs_tiles:
                        xs = xs_tiles[st]
                    else:
                        xs = xs_pool.tile([128, DM], f32, tag="xs", name=f"xs{st}")
                        nc.sync.dma_start(
                            out=xs, in_=x[st * 128 : (st + 1) * 128, :]
                        )
                    ssl = slice(st * 128, (st + 1) * 128)
                    for cg in range(2):  # groups of 4 contraction tiles
                        tp = tp_pool.tile([128, 512], f32, tag="tp")
                        for c4 in range(4):
                            c = cg * 4 + c4
                            nc.tensor.transpose(
                                tp[:, c4 * 128 : (c4 + 1) * 128],
                                xs[:, c * 128 : (c + 1) * 128],
                                identity,
                            )
                        nc.vector.tensor_copy(
                            xt[:, cg * 4 : (cg + 1) * 4, ssl],
                            tp[:].rearrange("p (c s) -> p c s", c=4),
                        )
                    # V projection for this s-tile (natural layout).
                    vps = proj_pool.tile([128, DKL], f32, tag="proj")
                    for c in range(NC_):
                        nc.tensor.matmul(
                            vps[:],
                            lhsT=xt[:, c, ssl],
                            rhs=wv_sb[:, c, :],
                            start=(c == 0),
                            stop=(c == NC_ - 1),
                            skip_group_check=True,
                        )
                    nc.vector.tensor_copy(
                        vaug[:, st, :, 0:HD],
                        vps[:].rearrange("p (h d) -> p h d", h=NHL),
                    )
                    # After each 512-row span: Q^T / K^T blocks for that span.
                    if st % 4 == 3:
                        q2 = st // 4
                        sl = slice(q2 * 512, (q2 + 1) * 512)
                        for wsb, dst in ((wq_sb, qt), (wk_sb, kt)):
                            for j in range(2):
                                ps = proj_pool.tile([128, 512], f32, tag="proj")
                                for c in range(NC_):
                                    nc.tensor.matmul(
                                        ps[:],
                                        lhsT=wsb[:, c, j * 128 : (j + 1) * 128],
                                        rhs=xt[:, c, sl],
                                        start=(c == 0),
                                        stop=(c == NC_ - 1),
                                        skip_group_check=True,
                                    )
                                nc.vector.tensor_copy(dst[j][:, sl], ps[:])

            # ---------------- Phase C: attention + output ------------------
            with (
                tc.tile_pool(name="sc_ps", bufs=3, space="PSUM") as sc_pool,
                tc.tile_pool(name="mm4_ps", bufs=2, space="PSUM") as mm4_pool,
                tc.tile_pool(name="pt", bufs=12) as pt_pool,
                tc.tile_pool(name="cc", bufs=8) as cc_pool,
                tc.tile_pool(name="rb", bufs=6) as rb_pool,
                tc.tile_pool(name="yo", bufs=4) as yo_pool,
            ):

                def emit_norm(h, mm4ps, c_tiles):
                    """concat^T[h rows] = heads^T / rowsum (deferred)."""
                    pt_i = h // 2
                    hp = (h % 2) * HD
                    recip_f = rb_pool.tile([1, 512], f32, tag="recipf", name=f"rcf{h}")
                    nc.vector.reciprocal(recip_f[:], mm4ps[HD : HD + 1, :])
                    recip = rb_pool.tile([1, 512], f32r, tag="recip", name=f"rc{h}")
                    nc.vector.tensor_copy(recip[:], recip_f[:])
                    bc = sc_pool.tile([HD, 512], f32, tag="sc", name=f"bc{h}")
                    nc.tensor.matmul(
                        bc[:],
                        lhsT=ones_t[:],
                        rhs=recip[:],
                        start=True,
                        stop=True,
                        skip_group_check=True,
                    )
                    rbt = rb_pool.tile([HD, 512], f32, tag="rb", name=f"rb{h}")
                    nc.vector.tensor_copy(rbt[:], bc[:])
                    nc.vector.tensor_mul(
                        c_tiles[pt_i][hp : hp + HD, :], mm4ps[0:HD, :], rbt[:]
                    )

                for q in range(NQC):
                    qsl = slice(q * 512, (q + 1) * 512)
                    c_tiles = [
                        cc_pool.tile([128, 512], f32r, tag="cc", name=f"cc{q}_{i}")
                        for i in range(2)
                    ]
                    pending = []
                    for pi in range(2):  # head pairs (0,1) and (2,3)
                        h0, h1 = 2 * pi, 2 * pi + 1
                        mm4 = [
                            mm4_pool.tile(
                                [HD + 1, 512], f32, tag="mm4", name=f"mm4_{q}_{h}"
                            )
                            for h in (h0, h1)
                        ]

                        def emit_mm4(hi, pt_g, g0, glen, mm4=mm4, pi=pi):
                            for j in range(glen):
                                t = g0 + j
                                nc.tensor.matmul(
                                    mm4[hi][:],
                                    lhsT=vaug[:, t, 2 * pi + hi, :],
                                    rhs=pt_g[hi][:, j * 512 : (j + 1) * 512],
                                    start=(t == 0),
                                    stop=(t == NST - 1),
                                    skip_group_check=True,
                                )

                        prev = None
                        for gi, (g0, glen) in enumerate(GROUPS2):
                            sc = [
                                sc_pool.tile(
                                    [128, glen * 512],
                                    f32,
                                    tag="sc",
                                    name=f"sc{q}_{pi}_{gi}_{i}",
                                )
                                for i in range(2)
                            ]
                            for j in range(glen):
                                t = g0 + j
                                tsl = slice(t * 128, (t + 1) * 128)
                                osl = slice(j * 512, (j + 1) * 512)
                                for hi, hpp in ((0, 0), (1, HD)):
                                    nc.tensor.matmul(
                                        sc[hi][:, osl],
                                        lhsT=kt[pi][hpp : hpp + HD, tsl],
                                        rhs=qt[pi][hpp : hpp + HD, qsl],
                                        start=True,
                                        stop=True,
                                        skip_group_check=True,
                                    )
                            pt_g = [
                                pt_pool.tile(
                                    [128, glen * 512],
                                    f32r,
                                    tag="pt",
                                    name=f"pt{q}_{pi}_{gi}_{i}",
                                )
                                for i in range(2)
                            ]
                            for hi in range(2):
                                nc.scalar.activation(
                                    out=pt_g[hi][:],
                                    in_=sc[hi][:],
                                    func=EXP,
                                    scale=0.125,
                                )
                            if prev is not None:
                                for hi in range(2):
                                    emit_mm4(hi, *prev)
                            if gi == 1 and pending:
                                for h, m in pending:
                                    emit_norm(h, m, c_tiles)
                                pending = []
                            prev = (pt_g, g0, glen)
                        for hi in range(2):
                            emit_mm4(hi, *prev)
                        pending = [(h0, mm4[0]), (h1, mm4[1])]
                    for h, m in pending:
                        emit_norm(h, m, c_tiles)

                    # Output projection for this 512-row span.
                    for sub in range(4):
                        yo = yo_pool.tile([128, DM], f32, tag="yo")
                        for nh2 in range(2):
                            ps5 = mm4_pool.tile(
                                [128, 512], f32, tag="mm4", name=f"ps5_{q}_{sub}_{nh2}"
                            )
                            for c2 in range(2):
                                nc.tensor.matmul(
                                    ps5[:],
                                    lhsT=c_tiles[c2][:, sub * 128 : (sub + 1) * 128],
                                    rhs=w0_sb[:, c2, nh2 * 512 : (nh2 + 1) * 512],
                                    start=(c2 == 0),
                                    stop=(c2 == 1),
                                    skip_group_check=True,
                                )
                            nc.vector.tensor_copy(
                                yo[:, nh2 * 512 : (nh2 + 1) * 512], ps5[:]
                            )
                        r0 = q * 512 + sub * 128
                        nc.sync.dma_start(out=y[r0 : r0 + 128, :], in_=yo[:])

    nc.compile()
    return nc


def _get_nc():
    if "nc" not in _CACHE:
        _CACHE["nc"] = _build_nc()
    return _CACHE["nc"]


def kernel(inputs, Wq, Wk, Wv, W0):
    global LAST_EXEC_NS
    from concourse.bass_utils import run_bass_kernel_spmd

    inputs = np.asarray(inputs, dtype=np.float32)
    Wq = np.asarray(Wq, dtype=np.float32)
    Wk = np.asarray(Wk, dtype=np.float32)
    Wv = np.asarray(Wv, dtype=np.float32)
    W0 = np.asarray(W0, dtype=np.float32)

    trace = bool(os.environ.get("BASS_KERNEL_TRACE"))
    if trace:
        trace = _maybe_enable_trace()

    nc = _get_nc()
    in_maps = []
    for c in range(8):
        b, g = divmod(c, 4)
        sl = slice(DKL * g, DKL * (g + 1))
        in_maps.append(
            {
                "x": np.ascontiguousarray(inputs[b]),
                "wq": np.ascontiguousarray(Wq[:, sl]),
                "wk": np.ascontiguousarray(Wk[:, sl]),
                "wv": np.ascontiguousarray(Wv[:, sl]),
                "w0": np.ascontiguousarray(W0[sl, :]),
            }
        )
    res = run_bass_kernel_spmd(nc, in_maps, list(range(8)), trace=trace)
    LAST_EXEC_NS = res.exec_time_ns
    outs = [res.results[i]["y"] for i in range(8)]
    out = np.stack(
        [
            outs[0] + outs[1] + outs[2] + outs[3],
            outs[4] + outs[5] + outs[6] + outs[7],
        ]
    )
    return out.astype(np.float32)
